# revision 1
# baseline (speedup 1.0000x reference)
"""Trainium2 Bass kernel for nn_ParabolicIntegrate.

Reference computation (per batch element b):
    dW[t]  = W[t] - W[t-1]            (dW[0] = 0)
    I[g][t] = sum_{s<=t} g[s] @ M^{t-s+1}   (causal block-Toeplitz "integral")
    f1 = I[dW]; f2 = I[f1^2]; f3 = I[f1^3]; f4 = I[dW*f1^2]
    out = stack([dW, f1, f2, f3, f4], axis=-1)    # [B, T, N, 5]

Sharding: pure data parallel over batch (64 -> 8 per core), M replicated.
Channel 0 (dW) is a pure data-movement channel; the host computes it during
input prep. The device computes the four integrals.

Device algorithm (per core, everything in column layout [N=128 part, T*B cols]):
  Three-level Toeplitz decomposition, no sequential scan. With L=4:
     W1_t  = sum_{l=1..4} g_{t-l+1} @ M^l          (4 matmuls, PSUM-accum)
     V_t   = W1_t + sum_{j=1..3} W1_{t-4j} @ M^{4j}   (3 matmuls)
     out_t = V_t  + sum_{i=1..3} V_{t-16i} @ M^{16i}  (3 matmuls)
  10 matmuls per integral, 40 total. Powers M^1..M^4, M^8, M^12, M^16,
  M^32, M^48 are host-precomputed (fp64 -> fp32, TF32-rounded). All matmul
  operands are float32r (TF32), accumulation fp32 in PSUM.
"""

import numpy as np

N = 128          # spatial points (= partition dim = contraction dim)
T = 64           # time points
B = 64           # total batch
NCORES = 8
BL = B // NCORES          # batch per core
NT = T * BL               # columns per core (t-major: col = t*BL + b)
C1 = 4                    # level-1 window (lags 1..4)
S1 = C1 * BL              # cols per level-1 stride (32)
S2 = C1 * C1 * BL         # cols per level-2 stride (128)
PAD = (C1 - 1) * BL       # front zero-pad for window reads (24)
W1LEN = NT - S1           # W1 cols read by combine-1 (480)
VLEN = NT - S2            # V cols read by combine-2 (384)
VPAD = 128                # front zero-pad of V copy (>=256 moving for f32r)
VSPLIT = 256              # first V-copy chunk (covers combine-2 i>=2)
NPOW = 9                  # M^1..M^4, M^8, M^12, M^16, M^32, M^48

_last_results = None      # BassKernelResults of the most recent run (for test.py)


def _make_tile_context(nc):
    """TileContext whose exit clears only the semaphores the kernel really
    used — the stock tail clears the allocator's whole ~100-sem pool one
    EVENT_SEMAPHORE at a time (several us of in-window tail)."""
    import concourse.tile as tile

    class LeanTileContext(tile.TileContext):
        def _drain_and_barrier(self, tick_clock, wait_clock):
            from concourse.vector_clock import ScopedClock

            drain_inst = self.nc.sync.drain()
            wait_clock.add_sem_waits(
                drain_inst.ins, ScopedClock({None: tick_clock.global_clock})
            )
            self.nc.all_engine_barrier()
            popped = self.nc._tile_sem_poison_stack.pop()
            assert popped is self._sem_poison
            used = set()
            for f in self.nc.m.functions:
                for b in f.blocks:
                    for i in b.instructions:
                        si = i.sync_info
                        if si is not None:
                            for w in (si.on_wait or []):
                                used.add(w.id)
                            for u in (si.on_update or []):
                                used.add(u.id)
            allocated = self.sems.allocated()
            clear = [s for s in allocated.values() if s.num in used]
            self.nc.clear_and_free_semaphores(clear)
            self.nc.all_engine_barrier()

    return LeanTileContext(nc)


def _build_bass():
    import concourse.bass as bass
    import concourse.mybir as mybir

    f32 = mybir.dt.float32
    f32r = mybir.dt.float32r

    nc = bass.Bass("TRN2", target_bir_lowering=False, debug=False,
                   num_devices=NCORES)

    dw_d = nc.dram_tensor("dWp", [N, PAD + NT], f32r, kind="ExternalInput").ap()
    pows_d = nc.dram_tensor("pows", [N, NPOW * N], f32r,
                            kind="ExternalInput").ap()
    # [N, 4, NT]: channels f1..f4; per-channel slices are per-partition
    # contiguous runs.
    out_d = nc.dram_tensor("out", [N, 4, NT], f32, kind="ExternalOutput").ap()

    with _make_tile_context(nc) as tc:
        with (
            tc.tile_pool(name="sbuf", bufs=1) as pool,
            tc.tile_pool(name="psum", bufs=1, space="PSUM") as psum,
        ):
            pows_s = pool.tile([N, NPOW * N], f32r, tag="pows_s")
            dWp = pool.tile([N, PAD + NT], f32r, tag="dWp")
            # Order: f1's window needs M^1..M^4 + dWp first; the combine
            # powers arrive while the window runs.
            nc.sync.dma_start(pows_s[:, 0:C1 * N], pows_d[:, 0:C1 * N])
            nc.sync.dma_start(dWp[:], dw_d[:, :])
            nc.sync.dma_start(pows_s[:, C1 * N:NPOW * N],
                              pows_d[:, C1 * N:NPOW * N])

            def pow_ap(i):
                return pows_s[:, i * N:(i + 1) * N]

            zeros = pool.tile([N, 256], f32, tag="zeros")
            nc.vector.memset(zeros[:], 0.0)

            def zero_pad(ap):
                nc.vector.tensor_copy(ap, zeros[:, 0:ap.shape[-1]])

            # HAM warmup: the PE clock sits at 1.2 GHz until ~3.4us of
            # sustained matmul activity. Burn that window on junk f32r
            # matmuls while the input DMAs run, so the real matmuls execute
            # at 2.4 GHz. `filler` is reused later to bridge PE-idle joints
            # (evacuation-copy latencies) so HAM never re-throttles.
            warm = pool.tile([N, NT], f32r, tag="warm")
            zero_pad(warm[:, 0:256])
            zero_pad(warm[:, 256:NT])
            wacc = psum.tile([N, NT], f32, tag="wacc")

            def filler(n):
                for _ in range(n):
                    nc.tensor.matmul(wacc[:, 0:NT], lhsT=warm[:, 0:N],
                                     rhs=warm[:, 0:NT], start=True, stop=True,
                                     skip_group_check=True)

            filler(6)
            # Preload the Scalar engine's Square activation table while the
            # DMAs run (first use of an ACT function loads its table, ~1us).
            sq_warm = pool.tile([N, 8], f32, tag="sq_warm")
            nc.scalar.activation(sq_warm[:], zeros[:, 0:8],
                                 mybir.ActivationFunctionType.Square)

            def window(acc, gp):
                """acc[:, t] = sum_{l=1..C1} gp_data[t-l+1] @ M^l."""
                for l in range(1, C1 + 1):
                    s0 = PAD - (l - 1) * BL
                    nc.tensor.matmul(
                        acc[:, 0:NT],
                        lhsT=pow_ap(l - 1),
                        rhs=gp[:, s0:s0 + NT],
                        start=(l == 1), stop=False, skip_group_check=True)

            def w1_copy(acc, name):
                w1 = pool.tile([N, W1LEN], f32r, tag=f"w1_{name}")
                nc.vector.tensor_copy(w1[:], acc[:, 0:W1LEN])
                return w1

            def combine1(acc, w1):
                """acc[:, t] += sum_{j=1..3} W1_{t-4j} @ M^{4j}."""
                for j in range(1, C1):
                    nc.tensor.matmul(
                        acc[:, j * S1:NT],
                        lhsT=pow_ap(2 + j),        # M^{4j}
                        rhs=w1[:, 0:NT - j * S1],
                        start=False, stop=False, skip_group_check=True)

            def v_copy(acc, name):
                """Evacuate V cols [0:VLEN], split so combine-2 i>=2 can
                start after the first chunk."""
                v = pool.tile([N, VPAD + VLEN], f32r, tag=f"v_{name}")
                zero_pad(v[:, 0:VPAD])
                nc.vector.tensor_copy(v[:, VPAD:VPAD + VSPLIT],
                                      acc[:, 0:VSPLIT])
                nc.vector.tensor_copy(v[:, VPAD + VSPLIT:VPAD + VLEN],
                                      acc[:, VSPLIT:VLEN])
                return v

            def combine2(acc, v):
                """acc[:, t] += sum_{i=1..3} V_{t-16i} @ M^{16i}.

                v has VPAD zero cols in front so every moving operand is
                >= 256 wide (float32r full-rate needs >=256). Emitted
                i=3..1: the high-i terms only need the first v chunk."""
                for i in range(C1 - 1, 0, -1):
                    L = max(NT - i * S2, 256)
                    o0 = NT - L
                    w0 = VPAD + o0 - i * S2
                    nc.tensor.matmul(
                        acc[:, o0:NT],
                        lhsT=pow_ap(5 + i),        # M^{16i}
                        rhs=v[:, w0:w0 + L],
                        start=False, stop=(i == 1), skip_group_check=True)

            # ---- f1 = I[dW] ----
            acc1 = psum.tile([N, NT], f32, tag="acc_f1")
            window(acc1, dWp)
            w1_1 = w1_copy(acc1, "f1")
            filler(2)          # keep PE busy while the W1 copy drains
            combine1(acc1, w1_1)
            v1 = v_copy(acc1, "f1")
            filler(2)          # ... and while the V copy drains
            combine2(acc1, v1)
            # All post-combine readers of the acc1 PSUM bank live on the
            # Scalar engine (sequential) — a concurrent DVE read of the same
            # bank would be a fatal PSUM collision (RAR is not tracked).
            g2p = pool.tile([N, PAD + NT], f32r, tag="g2p")
            g3p = pool.tile([N, PAD + NT], f32r, tag="g3p")
            g4p = pool.tile([N, PAD + NT], f32r, tag="g4p")
            for gp in (g2p, g3p, g4p):
                zero_pad(gp[:, 0:PAD])
            # f1^2 on Scalar (a TensorTensor may read only one PSUM input;
            # Square needs just one).
            nc.scalar.activation(g2p[:, PAD:PAD + NT], acc1[:, 0:NT],
                                 mybir.ActivationFunctionType.Square)
            f1_s = pool.tile([N, NT], f32, tag="f1_s")
            nc.scalar.copy(f1_s[:], acc1[:, 0:NT])
            nc.sync.dma_start(out_d[:, 0, :], f1_s[:])
            nc.vector.tensor_mul(g3p[:, PAD:PAD + NT],
                                 g2p[:, PAD:PAD + NT], f1_s[:])
            nc.vector.tensor_mul(g4p[:, PAD:PAD + NT],
                                 g2p[:, PAD:PAD + NT], dWp[:, PAD:PAD + NT])
            filler(3)          # bridge the square/integrand-prep latency

            # ---- f2, f3, f4 — phases interleaved so the PE never idles
            # while an evacuation copy (DVE) is in flight. ----
            acc2 = psum.tile([N, NT], f32, tag="acc_f2")
            acc3 = psum.tile([N, NT], f32, tag="acc_f3")
            acc4 = psum.tile([N, NT], f32, tag="acc_f4")

            window(acc2, g2p)
            window(acc3, g3p)
            w1_2 = w1_copy(acc2, "f2")
            combine1(acc2, w1_2)
            window(acc4, g4p)
            w1_3 = w1_copy(acc3, "f3")
            combine1(acc3, w1_3)
            v2 = v_copy(acc2, "f2")
            combine2(acc2, v2)
            w1_4 = w1_copy(acc4, "f4")
            combine1(acc4, w1_4)
            v3 = v_copy(acc3, "f3")
            f2_s = pool.tile([N, NT], f32, tag="fs_f2")
            nc.scalar.copy(f2_s[:], acc2[:, 0:NT])
            nc.sync.dma_start(out_d[:, 1, :], f2_s[:])
            combine2(acc3, v3)
            v4 = v_copy(acc4, "f4")
            f3_s = pool.tile([N, NT], f32, tag="fs_f3")
            nc.scalar.copy(f3_s[:], acc3[:, 0:NT])
            nc.sync.dma_start(out_d[:, 2, :], f3_s[:])
            filler(1)          # bridge f4's V-copy latency
            combine2(acc4, v4)
            # Last channel evacuates on DVE (faster than ACT; ACT may still
            # be copying f3 — different banks, so concurrent reads are fine).
            # Split in halves so the first DMA issues while the second half
            # is still copying — shortens the drain-on-last-DMA tail.
            f4_s = pool.tile([N, NT], f32, tag="fs_f4")
            nc.vector.tensor_copy(f4_s[:, 0:NT // 2], acc4[:, 0:NT // 2])
            nc.sync.dma_start(out_d[:, 3, 0:NT // 2], f4_s[:, 0:NT // 2])
            nc.vector.tensor_copy(f4_s[:, NT // 2:NT], acc4[:, NT // 2:NT])
            nc.sync.dma_start(out_d[:, 3, NT // 2:NT], f4_s[:, NT // 2:NT])

    _strip_entry_barrier(nc)
    _legalize_waits(nc)
    return nc


def _strip_entry_barrier(nc):
    """Remove bass's entry all-engine barrier (drain + EVSEM butterfly,
    ~1.5-2.5us) from the first block. It only orders the const-AP memsets
    against their consumers; our sole const consumer (Square bias) runs
    ~10us after the memsets, and the Square table-preload result is unused,
    so engines can enter the kernel unaligned."""
    import concourse.mybir as mybir

    blk = nc.m.functions[0].blocks[0]
    il = blk.instructions
    keep = [i for i in il
            if not isinstance(i, (mybir.InstDrain, mybir.InstEventSemaphore))]
    if len(keep) != len(il):
        il.clear()
        il.extend(keep)


def _legalize_waits(nc):
    """The walrus build here allows only ONE sync-wait per instruction.
    Tile emits instructions (and its final drain) with several. Split the
    extras into single-wait NOPs inserted just before, on the same engine —
    semantically identical (the engine blocks on each wait in sequence)."""
    import concourse.mybir as mybir

    n = 0
    for f in nc.m.functions:
        for b in f.blocks:
            il = b.instructions
            i = 0
            while i < len(il):
                inst = il[i]
                si = inst.sync_info
                if si is not None and si.on_wait and len(si.on_wait) > 1:
                    waits = list(si.on_wait)
                    for w in waits[:-1]:
                        n += 1
                        nop = mybir.InstNoOp(
                            name=f"I-waitsplit-{n}",
                            engine=inst.engine,
                            ins=[], outs=[],
                            sync_info=mybir.SyncInfo(on_wait=[w], on_update=[]),
                        )
                        il.insert(i, nop)
                        i += 1
                    inst.sync_info = mybir.SyncInfo(
                        on_wait=[waits[-1]],
                        on_update=list(si.on_update or []))
                i += 1
    return n


def _round_tf32(x):
    """Round fp32 array to TF32 (10 mantissa bits), round-to-nearest-even."""
    u = x.astype(np.float32).view(np.uint32)
    lsb = (u >> np.uint32(13)) & np.uint32(1)
    u = u + np.uint32(0xFFF) + lsb
    u = u & np.uint32(0xFFFFE000)
    return u.view(np.float32)


def _host_powers(M):
    M64 = M.astype(np.float64)
    P = {1: M64}
    for k in (2, 3, 4):
        P[k] = P[k - 1] @ M64
    P[8] = P[4] @ P[4]
    P[12] = P[8] @ P[4]
    P[16] = P[8] @ P[8]
    P[32] = P[16] @ P[16]
    P[48] = P[32] @ P[16]
    order = [1, 2, 3, 4, 8, 12, 16, 32, 48]
    assert len(order) == NPOW
    pows = np.concatenate([P[k].astype(np.float32) for k in order], axis=1)
    return np.ascontiguousarray(_round_tf32(pows))


def kernel(W, M):
    """W: [64, 64, 128] f32, M: [128, 128] f32 -> [64, 64, 128, 5] f32."""
    global _last_results
    import os
    from concourse.bass_utils import run_bass_kernel_spmd

    W = np.asarray(W, dtype=np.float32)
    M = np.asarray(M, dtype=np.float32)

    nc = _build_bass()

    pows_np = _host_powers(M)
    dW = np.zeros_like(W)                                 # [B, T, N] channel 0
    dW[:, 1:] = W[:, 1:] - W[:, :-1]

    in_maps = []
    for ci in range(NCORES):
        dw_col = np.ascontiguousarray(
            dW[ci * BL:(ci + 1) * BL].transpose(2, 1, 0).reshape(N, NT))
        dwp = np.zeros((N, PAD + NT), dtype=np.float32)
        dwp[:, PAD:] = _round_tf32(dw_col)
        in_maps.append({"dWp": dwp, "pows": pows_np})

    res = run_bass_kernel_spmd(nc, in_maps, core_ids=list(range(NCORES)),
                               trace=bool(os.environ.get("KERNEL_TRACE")))
    _last_results = res

    full = np.empty((B, T, N, 5), dtype=np.float32)
    full[..., 0] = dW
    for ci in range(NCORES):
        o = res.results[ci]["out"].reshape(N, 4, T, BL)
        full[ci * BL:(ci + 1) * BL, ..., 1:] = o.transpose(3, 2, 0, 1)
    return full



# revision 2
# speedup vs baseline: 1.1705x; 1.1705x over previous
"""Trainium2 Bass kernel for nn_ParabolicIntegrate.

Reference computation (per batch element b):
    dW[t]  = W[t] - W[t-1]            (dW[0] = 0)
    I[g][t] = sum_{l=1..t} g[t-l+1] @ M^l   (causal block-Toeplitz "integral")
    f1 = I[dW]; f2 = I[f1^2]; f3 = I[f1^3]; f4 = I[dW*f1^2]
    out = stack([dW, f1, f2, f3, f4], axis=-1)    # [B, T, N, 5]

Sharding: pure data parallel over batch (64 -> 8 per core), M replicated.
Channel 0 (dW) is computed host-side during input prep (pure data movement
channel); the device computes the four integrals.

Device algorithm (per core, column layout [N=128 part, NT=T*BL cols],
t-major: col = t*BL + b):
  Three-level Toeplitz decomposition, no sequential scan. With L=4:
     W1_t  = sum_{l=1..4} g_{t-l+1} @ M^l          (4 matmuls, PSUM-accum)
     V_t   = W1_t + sum_{j=1..3} W1_{t-4j} @ M^{4j}   (3 matmuls)
     out_t = V_t  + sum_{i=1..3} V_{t-16i} @ M^{16i}  (3 matmuls)
  10 matmuls per integral, 40 total, emitted with exact (shrinking) widths
  so no zero-padding of operands is needed anywhere.

Precision: fp16 operands (10-bit mantissa — same as TF32), fp32 PSUM
accumulation, fp16 intermediates and fp16 device output (host upcasts to
fp32).  Validated end-to-end in numpy: rel err ~1.9e-3 (gate is 2e-2).
fp16 halves all DMA bytes vs fp32/TF32 and enables FWL weight loads.

Schedule notes:
  - All three input DMAs issue immediately at kernel start from different
    engine queues (SP / ACT) so transfers overlap.
  - HAM warmup: junk matmuls on a zeroed tile keep the PE busy from ~0.2us
    so the 2.4 GHz clock unlocks (~3.4us of sustained PE activity) while
    the inputs stream in.
  - f1's chain is latency-critical; f2/f3/f4 windows+combines interleave
    so the PE never waits on an evacuation copy.
  - PSUM bank read rule: concurrent reads of one bank from two engines are
    fatal (RAR untracked).  Per-bank readers are serialized by true deps
    or same-engine ordering.
  - Tail: each channel is evacuated and DMA'd as soon as its combine2
    lands; DMA issues spread across SP/ACT/Pool queues.
  - No trailing all-engine barrier / semaphore clear: the NRT teardown
    that follows the kernel begins with its own all-engine barrier and
    clears every semaphore anyway; the kernel ends with just the final
    drain (which holds the last out-DMA completion waits).
"""

import numpy as np

N = 128          # spatial points (= partition dim = contraction dim)
T = 64           # time points
B = 64           # total batch
NCORES = 8
BL = B // NCORES          # batch per core
NT = T * BL               # columns per core (t-major: col = t*BL + b)
C1 = 4                    # level-1 window (lags 1..4)
S1 = C1 * BL              # cols per level-1 stride (32)
S2 = C1 * C1 * BL         # cols per level-2 stride (128)
W1LEN = NT - S1           # W1 cols read by combine-1 (480)
VLEN = NT - S2            # V cols read by combine-2 (384)
NPOW = 9                  # M^1..M^4, M^8, M^12, M^16, M^32, M^48

_last_results = None      # BassKernelResults of the most recent run (for test.py)


def _make_tile_context(nc):
    """TileContext whose exit emits ONLY the final drain (carrying the
    out-DMA completion waits).  The stock tail adds two all-engine barriers
    and clears every allocated semaphore one EVENT_SEMAPHORE at a time —
    all redundant here: the NRT teardown that runs right after the kernel
    starts with its own all-engine barrier and resets the whole semaphore
    file regardless."""
    import concourse.tile as tile

    class LeanTileContext(tile.TileContext):
        def _drain_and_barrier(self, tick_clock, wait_clock):
            from concourse.vector_clock import ScopedClock

            drain_inst = self.nc.sync.drain()
            wait_clock.add_sem_waits(
                drain_inst.ins, ScopedClock({None: tick_clock.global_clock})
            )
            popped = self.nc._tile_sem_poison_stack.pop()
            assert popped is self._sem_poison

    return LeanTileContext(nc)


def _build_bass():
    import concourse.bass as bass
    import concourse.mybir as mybir

    f16 = mybir.dt.float16
    f32 = mybir.dt.float32

    nc = bass.Bass("TRN2", target_bir_lowering=False, debug=False,
                   num_devices=NCORES)

    dw_d = nc.dram_tensor("dWh", [N, NT], f16, kind="ExternalInput").ap()
    pows_d = nc.dram_tensor("pows", [N, NPOW * N], f16,
                            kind="ExternalInput").ap()
    # [N, 4, NT]: channels f1..f4; per-channel slices are per-partition
    # contiguous runs.
    out_d = nc.dram_tensor("out", [N, 4, NT], f16, kind="ExternalOutput").ap()

    with _make_tile_context(nc) as tc:
        with (
            tc.tile_pool(name="sbuf", bufs=1) as pool,
            tc.tile_pool(name="psum", bufs=1, space="PSUM") as psum,
        ):
            pows_s = pool.tile([N, NPOW * N], f16, tag="pows_s")
            dWh = pool.tile([N, NT], f16, tag="dWh")
            warm = pool.tile([N, NT], f16, tag="warm")

            # t=0: all input DMAs in flight at once, from different queues.
            nc.sync.dma_start(dWh[:], dw_d[:, :])
            nc.scalar.dma_start(pows_s[:, 0:C1 * N], pows_d[:, 0:C1 * N])
            nc.sync.dma_start(pows_s[:, C1 * N:NPOW * N],
                              pows_d[:, C1 * N:NPOW * N])

            nc.vector.memset(warm[:], 0.0)

            # Preload the Scalar engine's Square activation table while the
            # DMAs run (first use of an ACT function loads its table, ~1us).
            sq_warm = pool.tile([N, 8], f16, tag="sq_warm")
            nc.scalar.activation(sq_warm[:], warm[:, 0:8],
                                 mybir.ActivationFunctionType.Square)

            def pow_ap(i):
                return pows_s[:, i * N:(i + 1) * N]

            # HAM warmup: the PE clock sits at 1.2 GHz until ~3.4us of
            # sustained matmul activity. Burn the input-DMA window on junk
            # fp16 matmuls so the real matmuls execute at 2.4 GHz. `filler`
            # is also used later to bridge PE-idle joints (evacuation-copy
            # latencies) so HAM never re-throttles.
            wacc = psum.tile([N, NT], f32, tag="wacc")

            def filler(n):
                for _ in range(n):
                    nc.tensor.matmul(wacc[:, 0:NT], lhsT=warm[:, 0:N],
                                     rhs=warm[:, 0:NT], start=True, stop=True,
                                     skip_group_check=True)

            filler(6)

            def window(acc, gp):
                """acc[:, t] = sum_{l=1..C1} gp[t-l+1] @ M^l, exact widths."""
                for l in range(1, C1 + 1):
                    s = (l - 1) * BL
                    nc.tensor.matmul(
                        acc[:, s:NT],
                        lhsT=pow_ap(l - 1),
                        rhs=gp[:, 0:NT - s],
                        start=(l == 1), stop=False, skip_group_check=True)

            def w1_copy(acc, name):
                w1 = pool.tile([N, W1LEN], f16, tag=f"w1_{name}")
                nc.vector.tensor_copy(w1[:], acc[:, 0:W1LEN])
                return w1

            def combine1(acc, w1):
                """acc[:, t] += sum_{j=1..3} W1_{t-4j} @ M^{4j}."""
                for j in range(1, C1):
                    nc.tensor.matmul(
                        acc[:, j * S1:NT],
                        lhsT=pow_ap(2 + j),        # M^{4j}
                        rhs=w1[:, 0:NT - j * S1],
                        start=False, stop=False, skip_group_check=True)

            def v_copy(acc, name):
                v = pool.tile([N, VLEN], f16, tag=f"v_{name}")
                nc.vector.tensor_copy(v[:], acc[:, 0:VLEN])
                return v

            def combine2(acc, v):
                """acc[:, t] += sum_{i=1..3} V_{t-16i} @ M^{16i}."""
                for i in range(C1 - 1, 0, -1):
                    nc.tensor.matmul(
                        acc[:, i * S2:NT],
                        lhsT=pow_ap(5 + i),        # M^{16i}
                        rhs=v[:, 0:NT - i * S2],
                        start=False, stop=(i == 1), skip_group_check=True)

            # ---- f1 = I[dW] ----
            acc1 = psum.tile([N, NT], f32, tag="acc_f1")
            window(acc1, dWh)
            w1_1 = w1_copy(acc1, "f1")
            filler(2)          # keep PE busy while the W1 copy drains
            combine1(acc1, w1_1)
            v1 = v_copy(acc1, "f1")
            filler(2)          # ... and while the V copy drains
            combine2(acc1, v1)

            # ---- integrand prep ----
            # acc1 bank readers are serialized: Square then copy, both on
            # ACT.  DVE touches only SBUF tiles here.
            g2p = pool.tile([N, NT], f16, tag="g2p")
            g3p = pool.tile([N, NT], f16, tag="g3p")
            g4p = pool.tile([N, NT], f16, tag="g4p")
            f1h = pool.tile([N, NT], f16, tag="f1h")
            nc.scalar.activation(g2p[:], acc1[:, 0:NT],
                                 mybir.ActivationFunctionType.Square)
            nc.scalar.copy(f1h[:], acc1[:, 0:NT])
            nc.sync.dma_start(out_d[:, 0, :], f1h[:])
            nc.vector.tensor_mul(g4p[:], g2p[:], dWh[:])
            nc.vector.tensor_mul(g3p[:], g2p[:], f1h[:])
            filler(3)          # bridge the integrand-prep latency

            # ---- f2, f3, f4 — windows/combines interleaved so the PE
            # never idles while an evacuation copy (DVE) is in flight. ----
            acc2 = psum.tile([N, NT], f32, tag="acc_f2")
            acc3 = psum.tile([N, NT], f32, tag="acc_f3")
            acc4 = psum.tile([N, NT], f32, tag="acc_f4")

            window(acc2, g2p)
            window(acc4, g4p)
            w1_2 = w1_copy(acc2, "f2")
            window(acc3, g3p)
            w1_4 = w1_copy(acc4, "f4")
            combine1(acc2, w1_2)
            w1_3 = w1_copy(acc3, "f3")
            combine1(acc4, w1_4)
            v2 = v_copy(acc2, "f2")
            combine1(acc3, w1_3)
            v4 = v_copy(acc4, "f4")
            combine2(acc2, v2)
            v3 = v_copy(acc3, "f3")
            # f2 out: ACT evacuates (DVE is busy with v3), SP issues.
            f2h = pool.tile([N, NT], f16, tag="f2h")
            nc.scalar.copy(f2h[:], acc2[:, 0:NT])
            nc.sync.dma_start(out_d[:, 1, :], f2h[:])
            combine2(acc4, v4)
            # f4 out: ACT evacuates, its own queue issues.
            f4h = pool.tile([N, NT], f16, tag="f4h")
            nc.scalar.copy(f4h[:], acc4[:, 0:NT])
            nc.scalar.dma_start(out_d[:, 3, :], f4h[:])
            combine2(acc3, v3)
            # f3 is last: DVE evacuates in halves so the first DMA issues
            # (on the Pool queue) while the second half is still copying.
            f3h = pool.tile([N, NT], f16, tag="f3h")
            nc.vector.tensor_copy(f3h[:, 0:NT // 2], acc3[:, 0:NT // 2])
            nc.gpsimd.dma_start(out_d[:, 2, 0:NT // 2], f3h[:, 0:NT // 2])
            nc.vector.tensor_copy(f3h[:, NT // 2:NT], acc3[:, NT // 2:NT])
            nc.gpsimd.dma_start(out_d[:, 2, NT // 2:NT], f3h[:, NT // 2:NT])

    _strip_entry_barrier(nc)
    _legalize_waits(nc)
    return nc


def _strip_entry_barrier(nc):
    """Remove bass's entry all-engine barrier (drain + EVSEM butterfly,
    ~1.5-2.5us) from the first block. It only orders the const-AP memsets
    against their consumers; our sole const consumer (Square bias) runs
    well after the memsets, so engines can enter the kernel unaligned."""
    import concourse.mybir as mybir

    blk = nc.m.functions[0].blocks[0]
    il = blk.instructions
    keep = [i for i in il
            if not isinstance(i, (mybir.InstDrain, mybir.InstEventSemaphore))]
    if len(keep) != len(il):
        il.clear()
        il.extend(keep)


def _legalize_waits(nc):
    """The walrus build here allows only ONE sync-wait per instruction.
    Tile emits instructions (and its final drain) with several. Split the
    extras into single-wait NOPs inserted just before, on the same engine —
    semantically identical (the engine blocks on each wait in sequence)."""
    import concourse.mybir as mybir

    n = 0
    for f in nc.m.functions:
        for b in f.blocks:
            il = b.instructions
            i = 0
            while i < len(il):
                inst = il[i]
                si = inst.sync_info
                if si is not None and si.on_wait and len(si.on_wait) > 1:
                    waits = list(si.on_wait)
                    for w in waits[:-1]:
                        n += 1
                        nop = mybir.InstNoOp(
                            name=f"I-waitsplit-{n}",
                            engine=inst.engine,
                            ins=[], outs=[],
                            sync_info=mybir.SyncInfo(on_wait=[w], on_update=[]),
                        )
                        il.insert(i, nop)
                        i += 1
                    inst.sync_info = mybir.SyncInfo(
                        on_wait=[waits[-1]],
                        on_update=list(si.on_update or []))
                i += 1
    return n


def _host_powers(M):
    M64 = M.astype(np.float64)
    P = {1: M64}
    for k in (2, 3, 4):
        P[k] = P[k - 1] @ M64
    P[8] = P[4] @ P[4]
    P[12] = P[8] @ P[4]
    P[16] = P[8] @ P[8]
    P[32] = P[16] @ P[16]
    P[48] = P[32] @ P[16]
    order = [1, 2, 3, 4, 8, 12, 16, 32, 48]
    assert len(order) == NPOW
    pows = np.concatenate([P[k] for k in order], axis=1)
    return np.ascontiguousarray(pows.astype(np.float16))


def kernel(W, M):
    """W: [64, 64, 128] f32, M: [128, 128] f32 -> [64, 64, 128, 5] f32."""
    global _last_results
    import os
    from concourse.bass_utils import run_bass_kernel_spmd

    W = np.asarray(W, dtype=np.float32)
    M = np.asarray(M, dtype=np.float32)

    nc = _build_bass()

    pows_np = _host_powers(M)
    dW = np.zeros_like(W)                                 # [B, T, N] channel 0
    dW[:, 1:] = W[:, 1:] - W[:, :-1]

    in_maps = []
    for ci in range(NCORES):
        dw_col = np.ascontiguousarray(
            dW[ci * BL:(ci + 1) * BL].transpose(2, 1, 0).reshape(N, NT))
        in_maps.append({"dWh": dw_col.astype(np.float16), "pows": pows_np})

    res = run_bass_kernel_spmd(nc, in_maps, core_ids=list(range(NCORES)),
                               trace=bool(os.environ.get("KERNEL_TRACE")))
    _last_results = res

    full = np.empty((B, T, N, 5), dtype=np.float32)
    full[..., 0] = dW
    for ci in range(NCORES):
        o = np.asarray(res.results[ci]["out"]).reshape(N, 4, T, BL)
        full[ci * BL:(ci + 1) * BL, ..., 1:] = \
            o.transpose(3, 2, 0, 1).astype(np.float32)
    return full
